# revision 1
# baseline (speedup 1.0000x reference)
"""Grouped-query attention (B=2, S=2048, H=2048, 16 q-heads / 4 kv-heads,
head_dim=128, QK-RMSNorm + RoPE) on 8 trn2 NeuronCores.

Sharding: core c = (batch b = c//4, kv-group g = c%4). Each core computes the
4 q-heads + 1 kv-head of its group for its batch, plus the partial o-proj
(contraction over its 512-row slice of Wo). Host sums the 4 group partials
per batch.

Device pipeline (layouts chosen so every big matmul is fp32r at 1 cyc/row):
  P1: QKV projection (lhsT = x^T tiles), fused RMSNorm + RoPE on Q/K in
      [s,d] layout, then PE-transpose Q,K -> Q^T,K^T ([d,s]).
  P2: per (head, q-chunk): scores^T[k,q] = K^T_tile.T @ Q^T (PSUM), exp via
      ACT (max-subtraction skipped: logits are O(5) for unit-RMS q/k, exp is
      safe in fp32), running row-sum accumulation on DVE, A*V computed as
      out^T[d,q] = V_tile.T @ expS^T. Softmax denominator applied via an
      all-ones matmul (column-sum broadcast to 128 partitions) + reciprocal.
  P3: o-proj: Y[q,:] += attnout^T_tile.T @ Wo_tile, evict + DMA out.
"""

import sys
from contextlib import ExitStack

import numpy as np

sys.path.insert(0, "/opt/trn_rl_repo")

import concourse.mybir as mybir  # noqa: E402
import concourse.tile as tile  # noqa: E402
from concourse import bacc  # noqa: E402
from concourse.bass_utils import run_bass_kernel_spmd  # noqa: E402

F32 = mybir.dt.float32
F32R = mybir.dt.float32r

B = 2
S = 2048
HIDDEN = 2048
NH = 16
NKV = 4
HD = 128
HPG = 4         # q-heads per core (one kv group)
ST = S // 128   # 16 s-tiles
HT = HIDDEN // 128  # 16 hidden tiles
EPS = 1e-6
SCALE = HD ** -0.5

_CACHE = {}


def build_nc():
    nc = bacc.Bacc("TRN2", target_bir_lowering=False, debug=False, num_devices=8)

    xt = nc.dram_tensor("xt", [ST, 128, HT, 128], F32R, kind="ExternalInput").ap()
    wqkv = nc.dram_tensor("wqkv", [128, HT, 768], F32R, kind="ExternalInput").ap()
    wo = nc.dram_tensor("wo", [128, HPG, HIDDEN], F32R, kind="ExternalInput").ap()
    cq = nc.dram_tensor("cq", [128, ST, HD], F32, kind="ExternalInput").ap()
    sq = nc.dram_tensor("sq", [128, ST, HD], F32, kind="ExternalInput").ap()
    ck = nc.dram_tensor("ck", [128, ST, HD], F32, kind="ExternalInput").ap()
    sk = nc.dram_tensor("sk", [128, ST, HD], F32, kind="ExternalInput").ap()
    ident = nc.dram_tensor("ident", [128, 128], F32R, kind="ExternalInput").ap()
    onesm = nc.dram_tensor("onesm", [128, 128], F32R, kind="ExternalInput").ap()
    y = nc.dram_tensor("y", [ST, 128, HIDDEN], F32, kind="ExternalOutput").ap()

    with tile.TileContext(nc) as tc:
        build_kernel(tc, xt, wqkv, wo, cq, sq, ck, sk, ident, onesm, y)
    nc.compile()
    return nc


def build_kernel(tc, xt, wqkv, wo, cq, sq, ck, sk, ident, onesm, y):
    nc = tc.nc
    Exp = mybir.ActivationFunctionType.Exp
    Sqrt = mybir.ActivationFunctionType.Sqrt
    Square = mybir.ActivationFunctionType.Square
    mult = mybir.AluOpType.mult
    add = mybir.AluOpType.add

    with ExitStack() as outer:
        const = outer.enter_context(tc.tile_pool(name="const", bufs=1))
        persist = outer.enter_context(tc.tile_pool(name="persist", bufs=1))

        id_sb = const.tile([128, 128], F32R)
        nc.sync.dma_start(id_sb[:], ident[:])
        ones_sb = const.tile([128, 128], F32R)
        nc.sync.dma_start(ones_sb[:], onesm[:])
        zb = const.tile([128, 1], F32)
        nc.vector.memset(zb[:], 0.0)
        epsb = const.tile([128, 1], F32)
        nc.vector.memset(epsb[:], EPS)

        qt_sb = persist.tile([128, HPG, S], F32R)     # Q^T per head [d, s]
        kt_sb = persist.tile([128, S], F32R)          # K^T [d, s]
        v_sb = persist.tile([128, ST, HD], F32R)      # V per s-tile [s, d]
        at0 = persist.tile([128, HPG, S // 2], F32R)  # attnout^T, q 0:1024
        at1 = persist.tile([128, HPG, S // 2], F32R)  # attnout^T, q 1024:2048

        # ---------------- Phase 1: QKV proj + RMSNorm + RoPE + transposes ----
        with (
            tc.tile_pool(name="p1c", bufs=1) as p1c,
            tc.tile_pool(name="p1x", bufs=3) as p1x,
            tc.tile_pool(name="p1ps", bufs=3, space="PSUM") as p1ps,
            tc.tile_pool(name="p1w", bufs=3) as p1w,
            tc.tile_pool(name="p1tp", bufs=2, space="PSUM") as p1tp,
        ):
            wqkv_sb = p1c.tile([128, HT, 768], F32R)
            cq_sb = p1c.tile([128, ST, HD], F32)
            sq_sb = p1c.tile([128, ST, HD], F32)
            ck_sb = p1c.tile([128, ST, HD], F32)
            sk_sb = p1c.tile([128, ST, HD], F32)

            # startup order: first x-tile, first weight chunks, trig, rest --
            # lets the first QKV matmuls start ~4us in instead of ~30us.
            xtile0 = p1x.tile([128, HT, 128], F32R, tag="xtile")
            nc.sync.dma_start(xtile0[:], xt[0])
            for t in range(HT):
                nc.sync.dma_start(wqkv_sb[:, t, :], wqkv[:, t, :])
            nc.gpsimd.dma_start(cq_sb[:], cq[:])
            nc.gpsimd.dma_start(sq_sb[:], sq[:])
            nc.gpsimd.dma_start(ck_sb[:], ck[:])
            nc.gpsimd.dma_start(sk_sb[:], sk[:])

            pend = None  # (rope_tile, i) with transposes not yet emitted

            def emit_transposes(rope_t, i0):
                for hh in range(5):
                    tp = p1tp.tile([128, 128], F32R)
                    nc.tensor.transpose(
                        tp[:], rope_t[:, hh * 128:(hh + 1) * 128], id_sb[:])
                    dst = (qt_sb[:, hh, i0 * 128:(i0 + 1) * 128] if hh < 4
                           else kt_sb[:, i0 * 128:(i0 + 1) * 128])
                    nc.scalar.copy(dst, tp[:])

            for i in range(ST):
                if i == 0:
                    xtile = xtile0
                else:
                    xtile = p1x.tile([128, HT, 128], F32R, tag="xtile")
                    nc.sync.dma_start(xtile[:], xt[i])
                qkv = p1ps.tile([128, 768], F32)
                for t in range(HT):
                    st, sp = (t == 0), (t == HT - 1)
                    nc.tensor.matmul(qkv[:, 0:512], (xtile[:, t, :]),
                                     (wqkv_sb[:, t, 0:512]), start=st, stop=sp)
                    nc.tensor.matmul(qkv[:, 512:768], (xtile[:, t, :]),
                                     (wqkv_sb[:, t, 512:768]), start=st, stop=sp)

                rope = p1w.tile([128, 640], F32R)
                scr = p1w.tile([128, 128], F32, tag="scr")
                stats = p1w.tile([128, 4], F32, tag="stats")
                for hh in range(5):  # 0..3 = q heads, 4 = k
                    off = hh * 128
                    cos = cq_sb if hh < 4 else ck_sb
                    sin = sq_sb if hh < 4 else sk_sb
                    # ssq on ACT (Square+accum); rms = sqrt(ssq/HD+eps)
                    nc.scalar.activation(scr[:], qkv[:, off:off + 128],
                                         Square, bias=zb[:],
                                         accum_out=stats[:, 0:1])
                    nc.scalar.activation(stats[:, 1:2], stats[:, 0:1], Sqrt,
                                         bias=epsb[:], scale=1.0 / HD)
                    nc.vector.reciprocal(stats[:, 2:3], stats[:, 1:2])
                    r = stats[:, 2:3]
                    # (q*r) .* cos   +   swap(q)*r .* sin  (sign/scale folded)
                    nc.vector.scalar_tensor_tensor(
                        scr[:], qkv[:, off:off + 128], r, cos[:, i, :], mult, mult)
                    nc.vector.scalar_tensor_tensor(
                        rope[:, off:off + 64], qkv[:, off + 64:off + 128], r,
                        sin[:, i, 0:64], mult, mult)
                    nc.vector.scalar_tensor_tensor(
                        rope[:, off + 64:off + 128], qkv[:, off:off + 64], r,
                        sin[:, i, 64:128], mult, mult)
                    nc.vector.tensor_add(rope[:, off:off + 128],
                                         rope[:, off:off + 128], scr[:])
                nc.scalar.copy(v_sb[:, i, :], qkv[:, 640:768])
                if pend is not None:
                    emit_transposes(*pend)
                pend = (rope, i)
            emit_transposes(*pend)

        # ---------------- Phase 2+3: attention with interleaved o-proj ----
        QC = 1024  # q-chunk
        with tc.tile_pool(name="p23c", bufs=1) as p23c:
            wo_sb = p23c.tile([128, HPG, HIDDEN], F32R)
            nc.sync.dma_start(wo_sb[:], wo[:])

            with (
                tc.tile_pool(name="scps", bufs=2, space="PSUM") as scps,
                tc.tile_pool(name="avps", bufs=1, space="PSUM") as avps,
                tc.tile_pool(name="exps", bufs=4) as exps,
                tc.tile_pool(name="sums", bufs=2) as sums_pool,
                tc.tile_pool(name="recs", bufs=2) as recs,
                tc.tile_pool(name="yps", bufs=2, space="PSUM") as yps,
                tc.tile_pool(name="ysb", bufs=3) as ysb_pool,
            ):
                def attention(h, qc):
                    q0 = qc * QC
                    at_q = at0 if qc == 0 else at1
                    sumsA = sums_pool.tile([128, QC], F32R, tag="sumsA")
                    sumsB = sums_pool.tile([128, QC], F32R, tag="sumsB")
                    avt = avps.tile([128, QC], F32)
                    for kt in range(ST):
                        sct = scps.tile([128, QC], F32)
                        for c in range(QC // 512):
                            csl = slice(c * 512, (c + 1) * 512)
                            nc.tensor.matmul(
                                sct[:, csl],
                                (kt_sb[:, kt * 128:(kt + 1) * 128]),
                                (qt_sb[:, h, q0 + c * 512:q0 + (c + 1) * 512]))
                        ex = exps.tile([128, QC], F32R)
                        nc.scalar.activation(ex[:], sct[:], Exp,
                                             bias=zb[:], scale=SCALE)
                        # running softmax-denominator adds split between
                        # DVE and GpSimd (6 of 16 on the slower GpSimd)
                        pool_turn = kt in (2, 4, 7, 9, 12, 14)
                        eng = nc.gpsimd if pool_turn else nc.vector
                        acc = sumsB if pool_turn else sumsA
                        first = (kt == 0) if not pool_turn else (kt == 2)
                        if first:
                            eng.tensor_copy(acc[:], ex[:])
                        else:
                            eng.tensor_add(acc[:], acc[:], ex[:])
                        for c in range(QC // 512):
                            csl = slice(c * 512, (c + 1) * 512)
                            nc.tensor.matmul(avt[:, csl], (v_sb[:, kt, :]),
                                             (ex[:, csl]),
                                             start=(kt == 0),
                                             stop=(kt == ST - 1))
                    nc.vector.tensor_add(sumsA[:], sumsA[:], sumsB[:])
                    bsum = scps.tile([128, QC], F32, tag="sct")
                    for c in range(QC // 512):
                        csl = slice(c * 512, (c + 1) * 512)
                        nc.tensor.matmul(bsum[:, csl], (ones_sb[:]),
                                         (sumsA[:, csl]))
                    rec = recs.tile([128, QC], F32)
                    for c in range(QC // 512):
                        csl = slice(c * 512, (c + 1) * 512)
                        nc.vector.reciprocal(rec[:, csl], bsum[:, csl])
                        nc.vector.tensor_mul(
                            at_q[:, h, c * 512:(c + 1) * 512],
                            avt[:, csl], rec[:, csl])

                def oproj(qt):
                    at_q = at0 if qt < 8 else at1
                    ytile = ysb_pool.tile([128, HIDDEN], F32)
                    for quarter in range(4):
                        yp = yps.tile([128, 512], F32)
                        osl = slice(quarter * 512, (quarter + 1) * 512)
                        for j in range(HPG):
                            nc.tensor.matmul(
                                yp[:],
                                (at_q[:, j, (qt % 8) * 128:(qt % 8 + 1) * 128]),
                                (wo_sb[:, j, osl]),
                                start=(j == 0), stop=(j == HPG - 1))
                        if quarter % 2 == 0:
                            nc.scalar.copy(ytile[:, osl], yp[:])
                        else:
                            nc.vector.tensor_copy(ytile[:, osl], yp[:])
                    nc.sync.dma_start(y[qt], ytile[:])

                for h in range(HPG):
                    attention(h, 0)
                for h in range(HPG):
                    attention(h, 1)
                    # at0 is complete: slot two o-proj q-tiles after each
                    # head so PE stays dense while ACT drains the exp backlog
                    oproj(2 * h)
                    oproj(2 * h + 1)
                for qt in range(8, ST):
                    oproj(qt)


def kernel(x, attention_mask, cos, sin, Wq, Wk, Wv, Wo, q_scale, k_scale):
    x = np.asarray(x, dtype=np.float32)
    cos = np.asarray(cos, dtype=np.float32)
    sin = np.asarray(sin, dtype=np.float32)
    Wq = np.asarray(Wq, dtype=np.float32)
    Wk = np.asarray(Wk, dtype=np.float32)
    Wv = np.asarray(Wv, dtype=np.float32)
    Wo = np.asarray(Wo, dtype=np.float32)
    q_scale = np.asarray(q_scale, dtype=np.float32)
    k_scale = np.asarray(k_scale, dtype=np.float32)

    if "nc" not in _CACHE:
        _CACHE["nc"] = build_nc()
    nc = _CACHE["nc"]

    sgn = np.concatenate([-np.ones(64, np.float32), np.ones(64, np.float32)])
    sigma = np.concatenate([np.arange(64, 128), np.arange(0, 64)])
    ident = np.eye(128, dtype=np.float32)
    onesm = np.ones((128, 128), dtype=np.float32)

    def tile_sd(a):
        # [S, 128] per-batch trig -> [128 s-part, ST, 128 d]
        return np.ascontiguousarray(
            a.reshape(ST, 128, HD).transpose(1, 0, 2)).astype(np.float32)

    in_maps = []
    for c in range(8):
        b, g = c // 4, c % 4
        xT = x[b].T  # [H, S]
        # per s-tile i the device wants sbuf [128 h-in-tile, HT, 128 s]
        xti = np.ascontiguousarray(
            xT.reshape(HT, 128, ST, 128).transpose(2, 1, 0, 3))
        wq_g = Wq[:, g * 512:(g + 1) * 512]
        wk_g = Wk[:, g * 128:(g + 1) * 128]
        wv_g = Wv[:, g * 128:(g + 1) * 128]
        wqkv = np.concatenate([wq_g, wk_g, wv_g], axis=1)  # [H, 768]
        wqkv = np.ascontiguousarray(
            wqkv.reshape(HT, 128, 768).transpose(1, 0, 2))  # [128, HT, 768]
        wo_g = Wo[g * 512:(g + 1) * 512, :]  # [512, H]
        wo_t = np.ascontiguousarray(
            wo_g.reshape(HPG, 128, HIDDEN).transpose(1, 0, 2))  # [128, 4, H]

        cosb, sinb = cos[b], sin[b]  # [S, 128]
        cq_h = cosb * q_scale[None, :]
        sq_h = (sinb * sgn[None, :]) * q_scale[sigma][None, :]
        ck_h = cosb * k_scale[None, :]
        sk_h = (sinb * sgn[None, :]) * k_scale[sigma][None, :]

        in_maps.append({
            "xt": xti.astype(np.float32),
            "wqkv": wqkv.astype(np.float32),
            "wo": wo_t.astype(np.float32),
            "cq": tile_sd(cq_h), "sq": tile_sd(sq_h),
            "ck": tile_sd(ck_h), "sk": tile_sd(sk_h),
            "ident": ident, "onesm": onesm,
        })

    res = run_bass_kernel_spmd(nc, in_maps, list(range(8)))
    outs = [r["y"].reshape(S, HIDDEN) for r in res.results]
    out = np.empty((B, S, HIDDEN), dtype=np.float32)
    for b in range(B):
        out[b] = outs[4 * b] + outs[4 * b + 1] + outs[4 * b + 2] + outs[4 * b + 3]
    return out



# revision 2
# speedup vs baseline: 1.0096x; 1.0096x over previous
"""Grouped-query attention (B=2, S=2048, H=2048, 16 q-heads / 4 kv-heads,
head_dim=128, QK-RMSNorm + RoPE) on 8 trn2 NeuronCores — v3 (all-bf16).

Sharding: core c = (batch b = c//4, kv-group g = c%4); host sums the 4 group
partials per batch.

vs v1 baseline:
  - every DMA'd tensor is bf16 (x, wqkv, wo, trig, y): half the HBM bytes;
    matmuls stay 1 cyc/row (bf16), accumulation in fp32 PSUM.
  - softmax: exp -> bf16, running denominator ladder entirely on DVE at the
    2x bf16 rate (Pool freed), partition-broadcast via bf16 ones matmul.
  - o-proj output evacuated as bf16 (DVE/Pool alternating) and DMA'd bf16.
  - RMSNorm: 5 ACT squares -> one batched ACT sqrt [128,5] -> one DVE recip.
  - startup: x tile 0 + 16 wqkv chunks alone on the sync queue; trig/ident/
    ones/wo on the gpsimd queue so weights stream just ahead of the matmuls.
  - tail: the last block (h=3, qc=1) runs as two 512-wide sub-chunks with
    o-proj quarters for q-tiles 8..11 interleaved into the final sub-chunk's
    kt loop; only q-tiles 12..15 remain after the last exp.
"""

import sys
from contextlib import ExitStack

import numpy as np
import ml_dtypes

sys.path.insert(0, "/opt/trn_rl_repo")

import concourse.mybir as mybir  # noqa: E402
import concourse.tile as tile  # noqa: E402
from concourse import bacc  # noqa: E402
from concourse.bass_utils import run_bass_kernel_spmd  # noqa: E402

F32 = mybir.dt.float32
F32R = mybir.dt.float32r
BF16 = mybir.dt.bfloat16

B = 2
S = 2048
HIDDEN = 2048
NH = 16
NKV = 4
HD = 128
HPG = 4          # q-heads per core (one kv group)
ST = S // 128    # 16 s-tiles
HT = HIDDEN // 128
EPS = 1e-6
SCALE = HD ** -0.5
QC = 1024        # attention q-chunk

_CACHE = {}


def build_nc():
    nc = bacc.Bacc("TRN2", target_bir_lowering=False, debug=False, num_devices=8)

    xt = nc.dram_tensor("xt", [ST, 128, HT, 128], BF16, kind="ExternalInput").ap()
    wqkv = nc.dram_tensor("wqkv", [128, HT, 768], BF16, kind="ExternalInput").ap()
    wo = nc.dram_tensor("wo", [128, HPG, HIDDEN], BF16, kind="ExternalInput").ap()
    cq = nc.dram_tensor("cq", [128, ST, HD], BF16, kind="ExternalInput").ap()
    sq = nc.dram_tensor("sq", [128, ST, HD], BF16, kind="ExternalInput").ap()
    ck = nc.dram_tensor("ck", [128, ST, HD], BF16, kind="ExternalInput").ap()
    sk = nc.dram_tensor("sk", [128, ST, HD], BF16, kind="ExternalInput").ap()
    identb = nc.dram_tensor("identb", [128, 128], F32R, kind="ExternalInput").ap()
    onesb = nc.dram_tensor("onesb", [128, 128], BF16, kind="ExternalInput").ap()
    y = nc.dram_tensor("y", [ST, 128, HIDDEN], BF16, kind="ExternalOutput").ap()

    with tile.TileContext(nc) as tc:
        build_kernel(tc, xt, wqkv, wo, cq, sq, ck, sk, identb, onesb, y)
    nc.compile()
    return nc


def build_kernel(tc, xt, wqkv, wo, cq, sq, ck, sk, identb, onesb, y):
    nc = tc.nc
    Exp = mybir.ActivationFunctionType.Exp
    Sqrt = mybir.ActivationFunctionType.Sqrt
    Square = mybir.ActivationFunctionType.Square
    mult = mybir.AluOpType.mult

    with ExitStack() as outer:
        const = outer.enter_context(tc.tile_pool(name="const", bufs=1))
        persist = outer.enter_context(tc.tile_pool(name="persist", bufs=1))

        id_sb = const.tile([128, 128], F32R)
        ones_sb = const.tile([128, 128], BF16)
        zb = const.tile([128, 1], F32)
        nc.vector.memset(zb[:], 0.0)
        epsb = const.tile([128, 1], F32)
        nc.vector.memset(epsb[:], EPS)

        qt_sb = persist.tile([128, HPG, S], BF16)     # Q^T per head [d, s]
        kt_sb = persist.tile([128, S], BF16)          # K^T [d, s]
        v_sb = persist.tile([128, ST, HD], BF16)      # V per s-tile [s, d]
        at0 = persist.tile([128, HPG, S // 2], BF16)  # attnout^T, q 0:1024
        at1 = persist.tile([128, HPG, S // 2], BF16)  # attnout^T, q 1024:2048
        wo_sb = persist.tile([128, HPG, HIDDEN], BF16)

        # ---------------- Phase 1: QKV proj + RMSNorm + RoPE + transposes ----
        with (
            tc.tile_pool(name="p1c", bufs=1) as p1c,
            tc.tile_pool(name="p1x", bufs=3) as p1x,
            tc.tile_pool(name="p1ps", bufs=3, space="PSUM") as p1ps,
            tc.tile_pool(name="p1w", bufs=4) as p1w,
            tc.tile_pool(name="p1tp", bufs=2, space="PSUM") as p1tp,
        ):
            wqkv_sb = p1c.tile([128, HT, 768], BF16)
            cq_sb = p1c.tile([128, ST, HD], BF16)
            sq_sb = p1c.tile([128, ST, HD], BF16)
            ck_sb = p1c.tile([128, ST, HD], BF16)
            sk_sb = p1c.tile([128, ST, HD], BF16)

            # sync queue: x tile 0 + weight chunks only, so the first QKV
            # matmul issues ~2us in; everything else on the gpsimd queue.
            xtile0 = p1x.tile([128, HT, 128], BF16, tag="xtile")
            nc.sync.dma_start(xtile0[:], xt[0])
            for t in range(HT):
                nc.sync.dma_start(wqkv_sb[:, t, :], wqkv[:, t, :])
            # gate the gpsimd DMA queue behind the weight stream so trig
            # /wo transfers don't interleave ahead of the wqkv chunks
            gate = p1c.tile([128, 1], BF16)
            nc.gpsimd.tensor_copy(gate[:], wqkv_sb[:, HT - 1, 0:1])
            nc.gpsimd.dma_start(id_sb[:], identb[:])
            nc.gpsimd.dma_start(cq_sb[:], cq[:])
            nc.gpsimd.dma_start(sq_sb[:], sq[:])
            nc.gpsimd.dma_start(ck_sb[:], ck[:])
            nc.gpsimd.dma_start(sk_sb[:], sk[:])
            nc.gpsimd.dma_start(ones_sb[:], onesb[:])
            nc.gpsimd.dma_start(wo_sb[:], wo[:])

            pend = []  # [(rope_tile, i)] with transposes not yet emitted

            def emit_transposes(rope_t, i0):
                for hh in range(5):
                    tp = p1tp.tile([128, 128], F32R)
                    nc.tensor.transpose(
                        tp[:], rope_t[:, hh * 128:(hh + 1) * 128], id_sb[:])
                    dst = (qt_sb[:, hh, i0 * 128:(i0 + 1) * 128] if hh < 4
                           else kt_sb[:, i0 * 128:(i0 + 1) * 128])
                    if hh % 2 == 0:
                        nc.scalar.copy(dst, tp[:])
                    else:
                        nc.vector.tensor_copy(dst, tp[:])

            for i in range(ST):
                if i == 0:
                    xtile = xtile0
                else:
                    xtile = p1x.tile([128, HT, 128], BF16, tag="xtile")
                    nc.sync.dma_start(xtile[:], xt[i])
                qkv = p1ps.tile([128, 768], F32)
                for t in range(HT):
                    st, sp = (t == 0), (t == HT - 1)
                    nc.tensor.matmul(qkv[:, 0:512], (xtile[:, t, :]),
                                     (wqkv_sb[:, t, 0:512]), start=st, stop=sp)
                    nc.tensor.matmul(qkv[:, 512:768], (xtile[:, t, :]),
                                     (wqkv_sb[:, t, 512:768]), start=st, stop=sp)

                rope = p1w.tile([128, 640], F32R)
                scr = p1w.tile([128, 128], F32, tag="scr")
                stats = p1w.tile([128, 15], F32, tag="stats")
                for hh in range(5):
                    nc.scalar.activation(scr[:], qkv[:, hh * 128:(hh + 1) * 128],
                                         Square, bias=zb[:],
                                         accum_out=stats[:, hh:hh + 1])
                nc.scalar.activation(stats[:, 5:10], stats[:, 0:5], Sqrt,
                                     bias=epsb[:], scale=1.0 / HD)
                nc.vector.reciprocal(stats[:, 10:15], stats[:, 5:10])
                for hh in range(5):  # 0..3 = q heads, 4 = k
                    off = hh * 128
                    cos = cq_sb if hh < 4 else ck_sb
                    sin = sq_sb if hh < 4 else sk_sb
                    r = stats[:, 10 + hh:11 + hh]
                    # (q*r) .* cos   +   swap(q)*r .* sin  (sign/scale folded)
                    nc.vector.scalar_tensor_tensor(
                        scr[:], qkv[:, off:off + 128], r, cos[:, i, :], mult, mult)
                    nc.vector.scalar_tensor_tensor(
                        rope[:, off:off + 64], qkv[:, off + 64:off + 128], r,
                        sin[:, i, 0:64], mult, mult)
                    nc.vector.scalar_tensor_tensor(
                        rope[:, off + 64:off + 128], qkv[:, off:off + 64], r,
                        sin[:, i, 64:128], mult, mult)
                    nc.vector.tensor_add(rope[:, off:off + 128],
                                         rope[:, off:off + 128], scr[:])
                nc.scalar.copy(v_sb[:, i, :], qkv[:, 640:768])
                pend.append((rope, i))
                if len(pend) > 2:
                    emit_transposes(*pend.pop(0))
            for p_ in pend:
                emit_transposes(*p_)

        # ---------------- Phase 2: attention + interleaved o-proj ------------
        with (
            tc.tile_pool(name="exps", bufs=4) as exps,
            tc.tile_pool(name="sums", bufs=2) as sums_pool,
            tc.tile_pool(name="recs", bufs=2) as recs,
            tc.tile_pool(name="ysb", bufs=4) as ysb_pool,
        ):
            scps = avps = trans = None

            def oproj_quarter(qt, quarter, pool_copy=False):
                at_q = at0 if qt < 8 else at1
                osl = slice(quarter * 512, (quarter + 1) * 512)
                yp = trans.tile([128, 512], F32, tag="yp")
                for j in range(HPG):
                    nc.tensor.matmul(
                        yp[:],
                        (at_q[:, j, (qt % 8) * 128:(qt % 8 + 1) * 128]),
                        (wo_sb[:, j, osl]),
                        start=(j == 0), stop=(j == HPG - 1))
                yb = ysb_pool.tile([128, 512], BF16)
                nc.vector.tensor_copy(yb[:], yp[:])
                nc.sync.dma_start(y[qt][:, osl], yb[:])

            def oproj(qt):
                for quarter in range(4):
                    oproj_quarter(qt, quarter, pool_copy=(quarter % 2 == 0))

            def emit_scores(h, q0, qw, kt):
                sct = scps.tile([128, qw], F32, tag="sct")
                for c in range(qw // 512):
                    nc.tensor.matmul(
                        sct[:, c * 512:(c + 1) * 512],
                        (kt_sb[:, kt * 128:(kt + 1) * 128]),
                        (qt_sb[:, h, q0 + c * 512:q0 + (c + 1) * 512]))
                return sct

            def block(h, q0, qw, filler=None, sct0=None, next_sc=None):
                # attention for head h over queries [q0, q0+qw)
                at_q, a0 = (at0, q0) if q0 < 1024 else (at1, q0 - 1024)
                nch = qw // 512
                avts = [avps.tile([128, 512], F32, tag="avt",
                                  name=f"avt{c}") for c in range(nch)]
                sumsA = sums_pool.tile([128, qw], BF16, tag="sumsA")

                # scores run one kt ahead of A@V so each exp's input is ready
                # before the previous exp retires (ACT back-to-back); the last
                # step emits the NEXT block's first scores before our AV drain
                sct = sct0 if sct0 is not None else emit_scores(h, q0, qw, 0)
                for kt in range(ST):
                    ex = exps.tile([128, qw], BF16, tag="ex")
                    nc.scalar.activation(ex[:], sct[:], Exp,
                                         bias=zb[:], scale=SCALE)
                    if kt + 1 < ST:
                        sct = emit_scores(h, q0, qw, kt + 1)
                    elif next_sc is not None:
                        next_sc()
                    if kt == 0:
                        nc.vector.tensor_copy(sumsA[:], ex[:])
                    else:
                        nc.vector.tensor_add(sumsA[:], sumsA[:], ex[:])
                    for c in range(nch):
                        csl = slice(c * 512, (c + 1) * 512)
                        nc.tensor.matmul(avts[c][:], (v_sb[:, kt, :]),
                                         (ex[:, csl]),
                                         start=(kt == 0),
                                         stop=(kt == ST - 1))
                    if filler is not None:
                        filler(kt)
                bsum = scps.tile([128, qw], F32, tag="sct")
                for c in range(nch):
                    csl = slice(c * 512, (c + 1) * 512)
                    nc.tensor.matmul(bsum[:, csl], (ones_sb[:]),
                                     (sumsA[:, csl]))
                rec = recs.tile([128, qw], F32)
                nc.vector.reciprocal(rec[:], bsum[:])
                for c in range(nch):
                    csl = slice(c * 512, (c + 1) * 512)
                    nc.vector.tensor_mul(at_q[:, h, a0 + c * 512:a0 + (c + 1) * 512],
                                         avts[c][:], rec[:, csl])

            with (
                tc.tile_pool(name="scpsU", bufs=2, space="PSUM") as scpsU,
                tc.tile_pool(name="avpsU", bufs=3, space="PSUM") as avpsU,
                tc.tile_pool(name="transU", bufs=1, space="PSUM") as transU,
            ):
                scps, avps, trans = scpsU, avpsU, transU
                units = [(0, 0, QC), (1, 0, QC), (2, 0, QC), (3, 0, QC),
                         (0, QC, QC), (1, QC, QC), (2, QC, QC),
                         (3, QC, 512), (3, QC + 512, 512)]
                fill = [(qt, quarter)
                        for qt in range(8, 12) for quarter in range(4)]

                def filler(kt):
                    qt, quarter = fill[kt]
                    oproj_quarter(qt, quarter, pool_copy=(kt % 2 == 0))

                hold = {}

                def make_next_sc(idx):
                    def f():
                        hn, qn, wn = units[idx]
                        hold["sct"] = emit_scores(hn, qn, wn, 0)
                    return f

                for idx, (hu, qu, wu) in enumerate(units):
                    sct0 = hold.pop("sct", None)
                    nxt = make_next_sc(idx + 1) if idx + 1 < len(units) else None
                    block(hu, qu, wu, sct0=sct0, next_sc=nxt,
                          filler=filler if idx == 8 else None)
                    if idx in (4, 5, 6):
                        hq = units[idx][0]
                        oproj(2 * hq)
                        oproj(2 * hq + 1)
                    elif idx == 7:
                        oproj(6)
                        oproj(7)

        with (
            tc.tile_pool(name="tailps", bufs=4, space="PSUM") as tailps,
            tc.tile_pool(name="tailsb", bufs=6) as tailsb,
        ):
            for qt in range(12, ST):
                for quarter in range(4):
                    osl = slice(quarter * 512, (quarter + 1) * 512)
                    yp = tailps.tile([128, 512], F32)
                    for j in range(HPG):
                        nc.tensor.matmul(
                            yp[:],
                            (at1[:, j, (qt % 8) * 128:(qt % 8 + 1) * 128]),
                            (wo_sb[:, j, osl]),
                            start=(j == 0), stop=(j == HPG - 1))
                    yb = tailsb.tile([128, 512], BF16)
                    if quarter % 2 == 0:
                        nc.scalar.copy(yb[:], yp[:])
                    else:
                        nc.vector.tensor_copy(yb[:], yp[:])
                    nc.sync.dma_start(y[qt][:, osl], yb[:])


def kernel(x, attention_mask, cos, sin, Wq, Wk, Wv, Wo, q_scale, k_scale):
    x = np.asarray(x, dtype=np.float32)
    cos = np.asarray(cos, dtype=np.float32)
    sin = np.asarray(sin, dtype=np.float32)
    Wq = np.asarray(Wq, dtype=np.float32)
    Wk = np.asarray(Wk, dtype=np.float32)
    Wv = np.asarray(Wv, dtype=np.float32)
    Wo = np.asarray(Wo, dtype=np.float32)
    q_scale = np.asarray(q_scale, dtype=np.float32)
    k_scale = np.asarray(k_scale, dtype=np.float32)

    if "nc" not in _CACHE:
        _CACHE["nc"] = build_nc()
    nc = _CACHE["nc"]

    bf16 = ml_dtypes.bfloat16
    sgn = np.concatenate([-np.ones(64, np.float32), np.ones(64, np.float32)])
    sigma = np.concatenate([np.arange(64, 128), np.arange(0, 64)])
    identb = np.eye(128, dtype=np.float32)
    onesb = np.ones((128, 128), dtype=np.float32).astype(bf16)

    def tile_sd(a):
        # [S, 128] per-batch trig -> [128 s-part, ST, 128 d]
        return np.ascontiguousarray(
            a.reshape(ST, 128, HD).transpose(1, 0, 2)).astype(bf16)

    in_maps = []
    for c in range(8):
        b, g = c // 4, c % 4
        xT = x[b].T  # [H, S]
        xti = np.ascontiguousarray(
            xT.reshape(HT, 128, ST, 128).transpose(2, 1, 0, 3)).astype(bf16)
        wq_g = Wq[:, g * 512:(g + 1) * 512]
        wk_g = Wk[:, g * 128:(g + 1) * 128]
        wv_g = Wv[:, g * 128:(g + 1) * 128]
        wqkv_g = np.concatenate([wq_g, wk_g, wv_g], axis=1)  # [H, 768]
        wqkv_g = np.ascontiguousarray(
            wqkv_g.reshape(HT, 128, 768).transpose(1, 0, 2)).astype(bf16)
        wo_g = Wo[g * 512:(g + 1) * 512, :]  # [512, H]
        wo_t = np.ascontiguousarray(
            wo_g.reshape(HPG, 128, HIDDEN).transpose(1, 0, 2)).astype(bf16)

        cosb, sinb = cos[b], sin[b]  # [S, 128]
        cq_h = cosb * q_scale[None, :]
        sq_h = (sinb * sgn[None, :]) * q_scale[sigma][None, :]
        ck_h = cosb * k_scale[None, :]
        sk_h = (sinb * sgn[None, :]) * k_scale[sigma][None, :]

        in_maps.append({
            "xt": xti,
            "wqkv": wqkv_g,
            "wo": wo_t,
            "cq": tile_sd(cq_h), "sq": tile_sd(sq_h),
            "ck": tile_sd(ck_h), "sk": tile_sd(sk_h),
            "identb": identb, "onesb": onesb,
        })

    res = run_bass_kernel_spmd(nc, in_maps, list(range(8)))
    outs = [r["y"].astype(np.float32).reshape(S, HIDDEN) for r in res.results]
    out = np.empty((B, S, HIDDEN), dtype=np.float32)
    for b in range(B):
        out[b] = (outs[4 * b] + outs[4 * b + 1]
                  + outs[4 * b + 2] + outs[4 * b + 3])
    return out


# revision 3
# speedup vs baseline: 1.0296x; 1.0199x over previous
"""Grouped-query attention (B=2, S=2048, H=2048, 16 q-heads / 4 kv-heads,
head_dim=128, QK-RMSNorm + RoPE) on 8 trn2 NeuronCores — v3 (all-bf16).

Sharding: core c = (batch b = c//4, kv-group g = c%4); host sums the 4 group
partials per batch.

vs v1 baseline:
  - every DMA'd tensor is bf16 (x, wqkv, wo, trig, y): half the HBM bytes;
    matmuls stay 1 cyc/row (bf16), accumulation in fp32 PSUM.
  - softmax: exp -> bf16, running denominator ladder entirely on DVE at the
    2x bf16 rate (Pool freed), partition-broadcast via bf16 ones matmul.
  - o-proj output evacuated as bf16 (DVE/Pool alternating) and DMA'd bf16.
  - RMSNorm: 5 ACT squares -> one batched ACT sqrt [128,5] -> one DVE recip.
  - startup: x tile 0 + 16 wqkv chunks alone on the sync queue; trig/ident/
    ones/wo on the gpsimd queue so weights stream just ahead of the matmuls.
  - tail: the last block (h=3, qc=1) runs as two 512-wide sub-chunks with
    o-proj quarters for q-tiles 8..11 interleaved into the final sub-chunk's
    kt loop; only q-tiles 12..15 remain after the last exp.
"""

import sys
from contextlib import ExitStack

import numpy as np
import ml_dtypes

sys.path.insert(0, "/opt/trn_rl_repo")

import concourse.mybir as mybir  # noqa: E402
import concourse.tile as tile  # noqa: E402
from concourse import bacc  # noqa: E402
from concourse.bass_utils import run_bass_kernel_spmd  # noqa: E402

F32 = mybir.dt.float32
F32R = mybir.dt.float32r
BF16 = mybir.dt.bfloat16

B = 2
S = 2048
HIDDEN = 2048
NH = 16
NKV = 4
HD = 128
HPG = 4          # q-heads per core (one kv group)
ST = S // 128    # 16 s-tiles
HT = HIDDEN // 128
EPS = 1e-6
SCALE = HD ** -0.5
QC = 1024        # attention q-chunk

_CACHE = {}


def build_nc():
    nc = bacc.Bacc("TRN2", target_bir_lowering=False, debug=False, num_devices=8)

    xt = nc.dram_tensor("xt", [ST, 128, HT, 128], BF16, kind="ExternalInput").ap()
    wqkv = nc.dram_tensor("wqkv", [128, HT, 768], BF16, kind="ExternalInput").ap()
    wo = nc.dram_tensor("wo", [128, HPG, HIDDEN], BF16, kind="ExternalInput").ap()
    cq = nc.dram_tensor("cq", [128, ST, HD], BF16, kind="ExternalInput").ap()
    sq = nc.dram_tensor("sq", [128, ST, HD], BF16, kind="ExternalInput").ap()
    ck = nc.dram_tensor("ck", [128, ST, HD], BF16, kind="ExternalInput").ap()
    sk = nc.dram_tensor("sk", [128, ST, HD], BF16, kind="ExternalInput").ap()
    identb = nc.dram_tensor("identb", [128, 128], BF16, kind="ExternalInput").ap()
    onesb = nc.dram_tensor("onesb", [128, 128], BF16, kind="ExternalInput").ap()
    y = nc.dram_tensor("y", [ST, 128, HIDDEN], BF16, kind="ExternalOutput").ap()

    with tile.TileContext(nc) as tc:
        build_kernel(tc, xt, wqkv, wo, cq, sq, ck, sk, identb, onesb, y)
    nc.compile()
    return nc


def build_kernel(tc, xt, wqkv, wo, cq, sq, ck, sk, identb, onesb, y):
    nc = tc.nc
    Exp = mybir.ActivationFunctionType.Exp
    Sqrt = mybir.ActivationFunctionType.Sqrt
    Square = mybir.ActivationFunctionType.Square
    mult = mybir.AluOpType.mult

    with ExitStack() as outer:
        const = outer.enter_context(tc.tile_pool(name="const", bufs=1))
        persist = outer.enter_context(tc.tile_pool(name="persist", bufs=1))

        id_sb = const.tile([128, 128], BF16)
        ones_sb = const.tile([128, 128], BF16)
        zb = const.tile([128, 1], F32)
        nc.vector.memset(zb[:], 0.0)
        epsb = const.tile([128, 1], F32)
        nc.vector.memset(epsb[:], EPS)

        qt_sb = persist.tile([128, HPG, S], BF16)     # Q^T per head [d, s]
        kt_sb = persist.tile([128, S], BF16)          # K^T [d, s]
        v_sb = persist.tile([128, ST, HD], BF16)      # V per s-tile [s, d]
        at0 = persist.tile([128, HPG, S // 2], BF16)  # attnout^T, q 0:1024
        at1 = persist.tile([128, HPG, S // 2], BF16)  # attnout^T, q 1024:2048
        wo_sb = persist.tile([128, HPG, HIDDEN], BF16)

        # ---------------- Phase 1: QKV proj + RMSNorm + RoPE + transposes ----
        with (
            tc.tile_pool(name="p1c", bufs=1) as p1c,
            tc.tile_pool(name="p1x", bufs=3) as p1x,
            tc.tile_pool(name="p1ps", bufs=3, space="PSUM") as p1ps,
            tc.tile_pool(name="p1w", bufs=4) as p1w,
            tc.tile_pool(name="p1tp", bufs=2, space="PSUM") as p1tp,
        ):
            wqkv_sb = p1c.tile([128, HT, 768], BF16)
            cq_sb = p1c.tile([128, ST, HD], BF16)
            sq_sb = p1c.tile([128, ST, HD], BF16)
            ck_sb = p1c.tile([128, ST, HD], BF16)
            sk_sb = p1c.tile([128, ST, HD], BF16)

            # sync queue: x tile 0 + weight chunks only, so the first QKV
            # matmul issues ~2us in; everything else on the gpsimd queue.
            xtile0 = p1x.tile([128, HT, 128], BF16, tag="xtile")
            nc.sync.dma_start(xtile0[:], xt[0])
            for t in range(HT):
                nc.sync.dma_start(wqkv_sb[:, t, :], wqkv[:, t, :])
            # gate the gpsimd DMA queue behind the weight stream so trig
            # /wo transfers don't interleave ahead of the wqkv chunks
            gate = p1c.tile([128, 1], BF16)
            nc.gpsimd.tensor_copy(gate[:], wqkv_sb[:, HT - 1, 0:1])
            nc.gpsimd.dma_start(id_sb[:], identb[:])
            nc.gpsimd.dma_start(cq_sb[:], cq[:])
            nc.gpsimd.dma_start(sq_sb[:], sq[:])
            nc.gpsimd.dma_start(ck_sb[:], ck[:])
            nc.gpsimd.dma_start(sk_sb[:], sk[:])
            nc.gpsimd.dma_start(ones_sb[:], onesb[:])
            nc.gpsimd.dma_start(wo_sb[:], wo[:])

            pend = []  # [(rope_tile, i)] with transposes not yet emitted

            def emit_transposes(rope_t, i0):
                for hh in range(5):
                    tp = p1tp.tile([128, 128], BF16)
                    nc.tensor.transpose(
                        tp[:], rope_t[:, hh * 128:(hh + 1) * 128], id_sb[:])
                    dst = (qt_sb[:, hh, i0 * 128:(i0 + 1) * 128] if hh < 4
                           else kt_sb[:, i0 * 128:(i0 + 1) * 128])
                    if hh % 2 == 0:
                        nc.scalar.copy(dst, tp[:])
                    else:
                        nc.vector.tensor_copy(dst, tp[:])

            for i in range(ST):
                if i == 0:
                    xtile = xtile0
                else:
                    xtile = p1x.tile([128, HT, 128], BF16, tag="xtile")
                    nc.sync.dma_start(xtile[:], xt[i])
                qkv = p1ps.tile([128, 768], F32)
                for t in range(HT):
                    st, sp = (t == 0), (t == HT - 1)
                    nc.tensor.matmul(qkv[:, 0:512], (xtile[:, t, :]),
                                     (wqkv_sb[:, t, 0:512]), start=st, stop=sp)
                    nc.tensor.matmul(qkv[:, 512:768], (xtile[:, t, :]),
                                     (wqkv_sb[:, t, 512:768]), start=st, stop=sp)

                rope = p1w.tile([128, 640], BF16)
                scr = p1w.tile([128, 128], BF16, tag="scr")
                stats = p1w.tile([128, 15], F32, tag="stats")
                for hh in range(5):
                    nc.scalar.activation(scr[:], qkv[:, hh * 128:(hh + 1) * 128],
                                         Square, bias=zb[:],
                                         accum_out=stats[:, hh:hh + 1])
                nc.scalar.activation(stats[:, 5:10], stats[:, 0:5], Sqrt,
                                     bias=epsb[:], scale=1.0 / HD)
                nc.vector.reciprocal(stats[:, 10:15], stats[:, 5:10])
                for hh in range(5):  # 0..3 = q heads, 4 = k
                    off = hh * 128
                    cos = cq_sb if hh < 4 else ck_sb
                    sin = sq_sb if hh < 4 else sk_sb
                    r = stats[:, 10 + hh:11 + hh]
                    # (q*r) .* cos   +   swap(q)*r .* sin  (sign/scale folded)
                    nc.vector.scalar_tensor_tensor(
                        scr[:], qkv[:, off:off + 128], r, cos[:, i, :], mult, mult)
                    nc.vector.scalar_tensor_tensor(
                        rope[:, off:off + 64], qkv[:, off + 64:off + 128], r,
                        sin[:, i, 0:64], mult, mult)
                    nc.vector.scalar_tensor_tensor(
                        rope[:, off + 64:off + 128], qkv[:, off:off + 64], r,
                        sin[:, i, 64:128], mult, mult)
                    nc.vector.tensor_add(rope[:, off:off + 128],
                                         rope[:, off:off + 128], scr[:])
                nc.scalar.copy(v_sb[:, i, :], qkv[:, 640:768])
                pend.append((rope, i))
                if len(pend) > 2:
                    emit_transposes(*pend.pop(0))
            for p_ in pend:
                emit_transposes(*p_)

        # ---------------- Phase 2: attention + interleaved o-proj ------------
        with (
            tc.tile_pool(name="exps", bufs=6) as exps,
            tc.tile_pool(name="sums", bufs=2) as sums_pool,
            tc.tile_pool(name="recs", bufs=2) as recs,
            tc.tile_pool(name="ysb", bufs=4) as ysb_pool,
        ):
            scps = avps = trans = None

            def oproj_quarter(qt, quarter, pool_copy=False):
                at_q = at0 if qt < 8 else at1
                osl = slice(quarter * 512, (quarter + 1) * 512)
                yp = trans.tile([128, 512], F32, tag="yp")
                for j in range(HPG):
                    nc.tensor.matmul(
                        yp[:],
                        (at_q[:, j, (qt % 8) * 128:(qt % 8 + 1) * 128]),
                        (wo_sb[:, j, osl]),
                        start=(j == 0), stop=(j == HPG - 1))
                yb = ysb_pool.tile([128, 512], BF16)
                nc.vector.tensor_copy(yb[:], yp[:])
                nc.sync.dma_start(y[qt][:, osl], yb[:])

            def oproj(qt):
                for quarter in range(4):
                    oproj_quarter(qt, quarter, pool_copy=(quarter % 2 == 0))

            def emit_scores(h, q0, qw, kt):
                sct = scps.tile([128, qw], F32, tag="sct")
                for c in range(qw // 512):
                    nc.tensor.matmul(
                        sct[:, c * 512:(c + 1) * 512],
                        (kt_sb[:, kt * 128:(kt + 1) * 128]),
                        (qt_sb[:, h, q0 + c * 512:q0 + (c + 1) * 512]))
                return sct

            def block(h, q0, qw, filler=None, sct0=None, next_sc=None):
                # attention for head h over queries [q0, q0+qw)
                at_q, a0 = (at0, q0) if q0 < 1024 else (at1, q0 - 1024)
                nch = qw // 512
                avts = [avps.tile([128, 512], F32, tag="avt",
                                  name=f"avt{c}") for c in range(nch)]
                sumsA = sums_pool.tile([128, qw], BF16, tag="sumsA")

                # scores run one kt ahead of A@V so each exp's input is ready
                # before the previous exp retires (ACT back-to-back); the last
                # step emits the NEXT block's first scores before our AV drain
                sct = sct0 if sct0 is not None else emit_scores(h, q0, qw, 0)
                for kt in range(ST):
                    ex = exps.tile([128, qw], BF16, tag="ex")
                    nc.scalar.activation(ex[:], sct[:], Exp,
                                         bias=zb[:], scale=SCALE)
                    if kt + 1 < ST:
                        sct = emit_scores(h, q0, qw, kt + 1)
                    elif next_sc is not None:
                        next_sc()
                    if kt == 0:
                        nc.vector.tensor_copy(sumsA[:], ex[:])
                    else:
                        nc.vector.tensor_add(sumsA[:], sumsA[:], ex[:])
                    for c in range(nch):
                        csl = slice(c * 512, (c + 1) * 512)
                        nc.tensor.matmul(avts[c][:], (v_sb[:, kt, :]),
                                         (ex[:, csl]),
                                         start=(kt == 0),
                                         stop=(kt == ST - 1))
                    if filler is not None:
                        filler(kt)
                bsum = scps.tile([128, qw], F32, tag="sct")
                for c in range(nch):
                    csl = slice(c * 512, (c + 1) * 512)
                    nc.tensor.matmul(bsum[:, csl], (ones_sb[:]),
                                     (sumsA[:, csl]))
                rec = recs.tile([128, qw], F32)
                nc.vector.reciprocal(rec[:], bsum[:])
                for c in range(nch):
                    csl = slice(c * 512, (c + 1) * 512)
                    nc.vector.tensor_mul(at_q[:, h, a0 + c * 512:a0 + (c + 1) * 512],
                                         avts[c][:], rec[:, csl])

            with (
                tc.tile_pool(name="scpsU", bufs=2, space="PSUM") as scpsU,
                tc.tile_pool(name="avpsU", bufs=3, space="PSUM") as avpsU,
                tc.tile_pool(name="transU", bufs=1, space="PSUM") as transU,
            ):
                scps, avps, trans = scpsU, avpsU, transU
                units = [(0, 0, QC), (1, 0, QC), (2, 0, QC), (3, 0, QC),
                         (0, QC, QC), (1, QC, QC), (2, QC, QC),
                         (3, QC, 512), (3, QC + 512, 512)]
                fill = [(qt, quarter)
                        for qt in range(8, 12) for quarter in range(4)]

                def filler(kt):
                    qt, quarter = fill[kt]
                    oproj_quarter(qt, quarter, pool_copy=(kt % 2 == 0))

                hold = {}

                def make_next_sc(idx):
                    def f():
                        hn, qn, wn = units[idx]
                        hold["sct"] = emit_scores(hn, qn, wn, 0)
                    return f

                for idx, (hu, qu, wu) in enumerate(units):
                    sct0 = hold.pop("sct", None)
                    nxt = make_next_sc(idx + 1) if idx + 1 < len(units) else None
                    block(hu, qu, wu, sct0=sct0, next_sc=nxt,
                          filler=filler if idx == 8 else None)
                    if idx in (4, 5, 6):
                        hq = units[idx][0]
                        oproj(2 * hq)
                        oproj(2 * hq + 1)
                    elif idx == 7:
                        oproj(6)
                        oproj(7)

        with (
            tc.tile_pool(name="tailps", bufs=4, space="PSUM") as tailps,
            tc.tile_pool(name="tailsb", bufs=6) as tailsb,
        ):
            for qt in range(12, ST):
                for quarter in range(4):
                    osl = slice(quarter * 512, (quarter + 1) * 512)
                    yp = tailps.tile([128, 512], F32)
                    for j in range(HPG):
                        nc.tensor.matmul(
                            yp[:],
                            (at1[:, j, (qt % 8) * 128:(qt % 8 + 1) * 128]),
                            (wo_sb[:, j, osl]),
                            start=(j == 0), stop=(j == HPG - 1))
                    yb = tailsb.tile([128, 512], BF16)
                    if quarter % 2 == 0:
                        nc.scalar.copy(yb[:], yp[:])
                    else:
                        nc.vector.tensor_copy(yb[:], yp[:])
                    nc.sync.dma_start(y[qt][:, osl], yb[:])


def kernel(x, attention_mask, cos, sin, Wq, Wk, Wv, Wo, q_scale, k_scale):
    x = np.asarray(x, dtype=np.float32)
    cos = np.asarray(cos, dtype=np.float32)
    sin = np.asarray(sin, dtype=np.float32)
    Wq = np.asarray(Wq, dtype=np.float32)
    Wk = np.asarray(Wk, dtype=np.float32)
    Wv = np.asarray(Wv, dtype=np.float32)
    Wo = np.asarray(Wo, dtype=np.float32)
    q_scale = np.asarray(q_scale, dtype=np.float32)
    k_scale = np.asarray(k_scale, dtype=np.float32)

    if "nc" not in _CACHE:
        _CACHE["nc"] = build_nc()
    nc = _CACHE["nc"]

    bf16 = ml_dtypes.bfloat16
    sgn = np.concatenate([-np.ones(64, np.float32), np.ones(64, np.float32)])
    sigma = np.concatenate([np.arange(64, 128), np.arange(0, 64)])
    identb = np.eye(128, dtype=np.float32).astype(bf16)
    onesb = np.ones((128, 128), dtype=np.float32).astype(bf16)

    def tile_sd(a):
        # [S, 128] per-batch trig -> [128 s-part, ST, 128 d]
        return np.ascontiguousarray(
            a.reshape(ST, 128, HD).transpose(1, 0, 2)).astype(bf16)

    in_maps = []
    for c in range(8):
        b, g = c // 4, c % 4
        xT = x[b].T  # [H, S]
        xti = np.ascontiguousarray(
            xT.reshape(HT, 128, ST, 128).transpose(2, 1, 0, 3)).astype(bf16)
        wq_g = Wq[:, g * 512:(g + 1) * 512]
        wk_g = Wk[:, g * 128:(g + 1) * 128]
        wv_g = Wv[:, g * 128:(g + 1) * 128]
        wqkv_g = np.concatenate([wq_g, wk_g, wv_g], axis=1)  # [H, 768]
        wqkv_g = np.ascontiguousarray(
            wqkv_g.reshape(HT, 128, 768).transpose(1, 0, 2)).astype(bf16)
        wo_g = Wo[g * 512:(g + 1) * 512, :]  # [512, H]
        wo_t = np.ascontiguousarray(
            wo_g.reshape(HPG, 128, HIDDEN).transpose(1, 0, 2)).astype(bf16)

        cosb, sinb = cos[b], sin[b]  # [S, 128]
        cq_h = cosb * q_scale[None, :]
        sq_h = (sinb * sgn[None, :]) * q_scale[sigma][None, :]
        ck_h = cosb * k_scale[None, :]
        sk_h = (sinb * sgn[None, :]) * k_scale[sigma][None, :]

        in_maps.append({
            "xt": xti,
            "wqkv": wqkv_g,
            "wo": wo_t,
            "cq": tile_sd(cq_h), "sq": tile_sd(sq_h),
            "ck": tile_sd(ck_h), "sk": tile_sd(sk_h),
            "identb": identb, "onesb": onesb,
        })

    res = run_bass_kernel_spmd(nc, in_maps, list(range(8)))
    outs = [r["y"].astype(np.float32).reshape(S, HIDDEN) for r in res.results]
    out = np.empty((B, S, HIDDEN), dtype=np.float32)
    for b in range(B):
        out[b] = (outs[4 * b] + outs[4 * b + 1]
                  + outs[4 * b + 2] + outs[4 * b + 3])
    return out


# revision 4
# speedup vs baseline: 1.0452x; 1.0151x over previous
"""Grouped-query attention (B=2, S=2048, H=2048, 16 q-heads / 4 kv-heads,
head_dim=128, QK-RMSNorm + RoPE) on 8 trn2 NeuronCores — v3 (all-bf16).

Sharding: core c = (batch b = c//4, kv-group g = c%4); host sums the 4 group
partials per batch.

vs v1 baseline:
  - every DMA'd tensor is bf16 (x, wqkv, wo, trig, y): half the HBM bytes;
    matmuls stay 1 cyc/row (bf16), accumulation in fp32 PSUM.
  - softmax: exp -> bf16, running denominator ladder entirely on DVE at the
    2x bf16 rate (Pool freed), partition-broadcast via bf16 ones matmul.
  - o-proj output evacuated as bf16 (DVE/Pool alternating) and DMA'd bf16.
  - RMSNorm: 5 ACT squares -> one batched ACT sqrt [128,5] -> one DVE recip.
  - startup: x tile 0 + 16 wqkv chunks alone on the sync queue; trig/ident/
    ones/wo on the gpsimd queue so weights stream just ahead of the matmuls.
  - tail: the last block (h=3, qc=1) runs as two 512-wide sub-chunks with
    o-proj quarters for q-tiles 8..11 interleaved into the final sub-chunk's
    kt loop; only q-tiles 12..15 remain after the last exp.
"""

import sys
from contextlib import ExitStack

import numpy as np
import ml_dtypes

sys.path.insert(0, "/opt/trn_rl_repo")

import concourse.mybir as mybir  # noqa: E402
import concourse.tile as tile  # noqa: E402
from concourse import bacc  # noqa: E402
from concourse.bass_utils import run_bass_kernel_spmd  # noqa: E402

F32 = mybir.dt.float32
F32R = mybir.dt.float32r
BF16 = mybir.dt.bfloat16

B = 2
S = 2048
HIDDEN = 2048
NH = 16
NKV = 4
HD = 128
HPG = 4          # q-heads per core (one kv group)
ST = S // 128    # 16 s-tiles
HT = HIDDEN // 128
EPS = 1e-6
SCALE = HD ** -0.5
QC = 1024        # attention q-chunk

_CACHE = {}


def build_nc():
    nc = bacc.Bacc("TRN2", target_bir_lowering=False, debug=False, num_devices=8)

    xt = nc.dram_tensor("xt", [ST, 128, HT, 128], BF16, kind="ExternalInput").ap()
    wqkv = nc.dram_tensor("wqkv", [128, HT, 768], BF16, kind="ExternalInput").ap()
    wo = nc.dram_tensor("wo", [128, HPG, HIDDEN], BF16, kind="ExternalInput").ap()
    cq = nc.dram_tensor("cq", [128, ST, HD], BF16, kind="ExternalInput").ap()
    sq = nc.dram_tensor("sq", [128, ST, HD], BF16, kind="ExternalInput").ap()
    ck = nc.dram_tensor("ck", [128, ST, HD], BF16, kind="ExternalInput").ap()
    sk = nc.dram_tensor("sk", [128, ST, HD], BF16, kind="ExternalInput").ap()
    identb = nc.dram_tensor("identb", [128, 128], BF16, kind="ExternalInput").ap()
    onesb = nc.dram_tensor("onesb", [128, 128], BF16, kind="ExternalInput").ap()
    y = nc.dram_tensor("y", [ST, 128, HIDDEN], BF16, kind="ExternalOutput").ap()

    with tile.TileContext(nc) as tc:
        build_kernel(tc, xt, wqkv, wo, cq, sq, ck, sk, identb, onesb, y)
    nc.compile()
    return nc


def build_kernel(tc, xt, wqkv, wo, cq, sq, ck, sk, identb, onesb, y):
    nc = tc.nc
    Exp = mybir.ActivationFunctionType.Exp
    Sqrt = mybir.ActivationFunctionType.Sqrt
    Square = mybir.ActivationFunctionType.Square
    mult = mybir.AluOpType.mult

    with ExitStack() as outer:
        const = outer.enter_context(tc.tile_pool(name="const", bufs=1))
        persist = outer.enter_context(tc.tile_pool(name="persist", bufs=1))

        id_sb = const.tile([128, 128], BF16)
        ones_sb = const.tile([128, 128], BF16)
        zb = const.tile([128, 1], F32)
        nc.vector.memset(zb[:], 0.0)
        epsb = const.tile([128, 1], F32)
        nc.vector.memset(epsb[:], EPS)

        qt_sb = persist.tile([128, HPG, S], BF16)     # Q^T per head [d, s]
        kt_sb = persist.tile([128, S], BF16)          # K^T [d, s]
        v_sb = persist.tile([128, ST, HD], BF16)      # V per s-tile [s, d]
        at0 = persist.tile([128, HPG, S // 2], BF16)  # attnout^T, q 0:1024
        at1 = persist.tile([128, HPG, S // 2], BF16)  # attnout^T, q 1024:2048
        wo_sb = persist.tile([128, HPG, HIDDEN], BF16)

        # ---------------- Phase 1: QKV proj + RMSNorm + RoPE + transposes ----
        with (
            tc.tile_pool(name="p1c", bufs=1) as p1c,
            tc.tile_pool(name="p1x", bufs=3) as p1x,
            tc.tile_pool(name="p1ps", bufs=3, space="PSUM") as p1ps,
            tc.tile_pool(name="p1w", bufs=4) as p1w,
            tc.tile_pool(name="p1tp", bufs=2, space="PSUM") as p1tp,
        ):
            wqkv_sb = p1c.tile([128, HT, 768], BF16)
            cq_sb = p1c.tile([128, ST, HD], BF16)
            sq_sb = p1c.tile([128, ST, HD], BF16)
            ck_sb = p1c.tile([128, ST, HD], BF16)
            sk_sb = p1c.tile([128, ST, HD], BF16)

            # sync queue: x tile 0 + weight chunks only, so the first QKV
            # matmul issues ~2us in; everything else on the gpsimd queue.
            xtile0 = p1x.tile([128, HT, 128], BF16, tag="xtile")
            nc.sync.dma_start(xtile0[:], xt[0])
            for t in range(HT):
                nc.sync.dma_start(wqkv_sb[:, t, :], wqkv[:, t, :])
            # gate the gpsimd DMA queue behind the weight stream so trig
            # /wo transfers don't interleave ahead of the wqkv chunks
            gate = p1c.tile([128, 1], BF16)
            nc.gpsimd.tensor_copy(gate[:], wqkv_sb[:, HT - 1, 0:1])
            nc.gpsimd.dma_start(id_sb[:], identb[:])
            nc.gpsimd.dma_start(cq_sb[:], cq[:])
            nc.gpsimd.dma_start(sq_sb[:], sq[:])
            nc.gpsimd.dma_start(ck_sb[:], ck[:])
            nc.gpsimd.dma_start(sk_sb[:], sk[:])
            nc.gpsimd.dma_start(ones_sb[:], onesb[:])
            nc.gpsimd.dma_start(wo_sb[:], wo[:])

            pend = []  # [(rope_tile, i)] with transposes not yet emitted

            def emit_transposes(rope_t, i0):
                for hh in range(5):
                    tp = p1tp.tile([128, 128], BF16)
                    nc.tensor.transpose(
                        tp[:], rope_t[:, hh * 128:(hh + 1) * 128], id_sb[:])
                    dst = (qt_sb[:, hh, i0 * 128:(i0 + 1) * 128] if hh < 4
                           else kt_sb[:, i0 * 128:(i0 + 1) * 128])
                    if hh % 2 == 0:
                        nc.scalar.copy(dst, tp[:])
                    else:
                        nc.vector.tensor_copy(dst, tp[:])

            for i in range(ST):
                if i == 0:
                    xtile = xtile0
                else:
                    xtile = p1x.tile([128, HT, 128], BF16, tag="xtile")
                    nc.sync.dma_start(xtile[:], xt[i])
                qkv = p1ps.tile([128, 768], F32)
                for t in range(HT):
                    st, sp = (t == 0), (t == HT - 1)
                    nc.tensor.matmul(qkv[:, 0:512], (xtile[:, t, :]),
                                     (wqkv_sb[:, t, 0:512]), start=st, stop=sp)
                    nc.tensor.matmul(qkv[:, 512:768], (xtile[:, t, :]),
                                     (wqkv_sb[:, t, 512:768]), start=st, stop=sp)

                rope = p1w.tile([128, 640], BF16)
                scr = p1w.tile([128, 128], BF16, tag="scr")
                stats = p1w.tile([128, 15], F32, tag="stats")
                for hh in range(5):
                    nc.scalar.activation(scr[:], qkv[:, hh * 128:(hh + 1) * 128],
                                         Square, bias=zb[:],
                                         accum_out=stats[:, hh:hh + 1])
                nc.scalar.activation(stats[:, 5:10], stats[:, 0:5], Sqrt,
                                     bias=epsb[:], scale=1.0 / HD)
                nc.vector.reciprocal(stats[:, 10:15], stats[:, 5:10])
                for hh in range(5):  # 0..3 = q heads, 4 = k
                    off = hh * 128
                    cos = cq_sb if hh < 4 else ck_sb
                    sin = sq_sb if hh < 4 else sk_sb
                    r = stats[:, 10 + hh:11 + hh]
                    # (q*r) .* cos   +   swap(q)*r .* sin  (sign/scale folded)
                    nc.vector.scalar_tensor_tensor(
                        scr[:], qkv[:, off:off + 128], r, cos[:, i, :], mult, mult)
                    nc.vector.scalar_tensor_tensor(
                        rope[:, off:off + 64], qkv[:, off + 64:off + 128], r,
                        sin[:, i, 0:64], mult, mult)
                    nc.vector.scalar_tensor_tensor(
                        rope[:, off + 64:off + 128], qkv[:, off:off + 64], r,
                        sin[:, i, 64:128], mult, mult)
                    nc.vector.tensor_add(rope[:, off:off + 128],
                                         rope[:, off:off + 128], scr[:])
                nc.scalar.copy(v_sb[:, i, :], qkv[:, 640:768])
                pend.append((rope, i))
                if len(pend) > 2:
                    emit_transposes(*pend.pop(0))
            for p_ in pend:
                emit_transposes(*p_)

        # ---------------- Phase 2: attention + interleaved o-proj ------------
        with (
            tc.tile_pool(name="exps", bufs=6) as exps,
            tc.tile_pool(name="sums", bufs=2) as sums_pool,
            tc.tile_pool(name="recs", bufs=2) as recs,
            tc.tile_pool(name="ysb", bufs=4) as ysb_pool,
        ):
            scps = avps = trans = None

            def oproj_quarter(qt, quarter, pool_copy=False):
                at_q = at0 if qt < 8 else at1
                osl = slice(quarter * 512, (quarter + 1) * 512)
                yp = trans.tile([128, 512], F32, tag="yp")
                for j in range(HPG):
                    nc.tensor.matmul(
                        yp[:],
                        (at_q[:, j, (qt % 8) * 128:(qt % 8 + 1) * 128]),
                        (wo_sb[:, j, osl]),
                        start=(j == 0), stop=(j == HPG - 1))
                yb = ysb_pool.tile([128, 512], BF16)
                nc.vector.tensor_copy(yb[:], yp[:])
                nc.sync.dma_start(y[qt][:, osl], yb[:])

            def oproj(qt):
                for quarter in range(4):
                    oproj_quarter(qt, quarter, pool_copy=(quarter % 2 == 0))

            def emit_scores(h, q0, qw, kt):
                sct = scps.tile([128, qw], F32, tag="sct")
                for c in range(qw // 512):
                    nc.tensor.matmul(
                        sct[:, c * 512:(c + 1) * 512],
                        (kt_sb[:, kt * 128:(kt + 1) * 128]),
                        (qt_sb[:, h, q0 + c * 512:q0 + (c + 1) * 512]))
                return sct

            def block(h, q0, qw, filler=None, sct0=None, next_sc=None):
                # attention for head h over queries [q0, q0+qw)
                at_q, a0 = (at0, q0) if q0 < 1024 else (at1, q0 - 1024)
                nch = qw // 512
                avts = [avps.tile([128, 512], F32, tag="avt",
                                  name=f"avt{c}") for c in range(nch)]
                sumsA = sums_pool.tile([128, qw], BF16, tag="sumsA")

                # scores run one kt ahead of A@V so each exp's input is ready
                # before the previous exp retires (ACT back-to-back); the last
                # step emits the NEXT block's first scores before our AV drain
                sct = sct0 if sct0 is not None else emit_scores(h, q0, qw, 0)
                for kt in range(ST):
                    ex = exps.tile([128, qw], BF16, tag="ex")
                    nc.scalar.activation(ex[:], sct[:], Exp,
                                         bias=zb[:], scale=SCALE)
                    if kt + 1 < ST:
                        sct = emit_scores(h, q0, qw, kt + 1)
                    elif next_sc is not None:
                        next_sc()
                    if kt == 0:
                        nc.vector.tensor_copy(sumsA[:], ex[:])
                    else:
                        nc.vector.tensor_add(sumsA[:], sumsA[:], ex[:])
                    for c in range(nch):
                        csl = slice(c * 512, (c + 1) * 512)
                        nc.tensor.matmul(avts[c][:], (v_sb[:, kt, :]),
                                         (ex[:, csl]),
                                         start=(kt == 0),
                                         stop=(kt == ST - 1))
                    if filler is not None:
                        filler(kt)
                bsum = scps.tile([128, qw], F32, tag="sct")
                for c in range(nch):
                    csl = slice(c * 512, (c + 1) * 512)
                    nc.tensor.matmul(bsum[:, csl], (ones_sb[:]),
                                     (sumsA[:, csl]))
                rec = recs.tile([128, qw], F32)
                nc.vector.reciprocal(rec[:], bsum[:])
                for c in range(nch):
                    csl = slice(c * 512, (c + 1) * 512)
                    nc.vector.tensor_mul(at_q[:, h, a0 + c * 512:a0 + (c + 1) * 512],
                                         avts[c][:], rec[:, csl])

            with (
                tc.tile_pool(name="scpsU", bufs=2, space="PSUM") as scpsU,
                tc.tile_pool(name="avpsU", bufs=2, space="PSUM") as avpsU,
                tc.tile_pool(name="transU", bufs=2, space="PSUM") as transU,
            ):
                scps, avps, trans = scpsU, avpsU, transU
                units = [(0, 0, QC), (1, 0, QC), (2, 0, QC), (3, 0, QC),
                         (0, QC, QC), (1, QC, QC), (2, QC, QC),
                         (3, QC, 512), (3, QC + 512, 512)]
                fill = [(qt, quarter)
                        for qt in range(8, 12) for quarter in range(4)]

                def filler(kt):
                    qt, quarter = fill[kt]
                    oproj_quarter(qt, quarter, pool_copy=(kt % 2 == 0))

                hold = {}

                def make_next_sc(idx):
                    def f():
                        hn, qn, wn = units[idx]
                        hold["sct"] = emit_scores(hn, qn, wn, 0)
                    return f

                for idx, (hu, qu, wu) in enumerate(units):
                    sct0 = hold.pop("sct", None)
                    nxt = make_next_sc(idx + 1) if idx + 1 < len(units) else None
                    block(hu, qu, wu, sct0=sct0, next_sc=nxt,
                          filler=filler if idx == 8 else None)
                    if idx in (4, 5, 6):
                        hq = units[idx][0]
                        oproj(2 * hq)
                        oproj(2 * hq + 1)
                    elif idx == 7:
                        oproj(6)
                        oproj(7)

        with (
            tc.tile_pool(name="tailps", bufs=4, space="PSUM") as tailps,
            tc.tile_pool(name="tailsb", bufs=6) as tailsb,
        ):
            for qt in range(12, ST):
                for quarter in range(4):
                    osl = slice(quarter * 512, (quarter + 1) * 512)
                    yp = tailps.tile([128, 512], F32)
                    for j in range(HPG):
                        nc.tensor.matmul(
                            yp[:],
                            (at1[:, j, (qt % 8) * 128:(qt % 8 + 1) * 128]),
                            (wo_sb[:, j, osl]),
                            start=(j == 0), stop=(j == HPG - 1))
                    yb = tailsb.tile([128, 512], BF16)
                    if quarter % 2 == 0:
                        nc.scalar.copy(yb[:], yp[:])
                    else:
                        nc.vector.tensor_copy(yb[:], yp[:])
                    nc.sync.dma_start(y[qt][:, osl], yb[:])


def kernel(x, attention_mask, cos, sin, Wq, Wk, Wv, Wo, q_scale, k_scale):
    x = np.asarray(x, dtype=np.float32)
    cos = np.asarray(cos, dtype=np.float32)
    sin = np.asarray(sin, dtype=np.float32)
    Wq = np.asarray(Wq, dtype=np.float32)
    Wk = np.asarray(Wk, dtype=np.float32)
    Wv = np.asarray(Wv, dtype=np.float32)
    Wo = np.asarray(Wo, dtype=np.float32)
    q_scale = np.asarray(q_scale, dtype=np.float32)
    k_scale = np.asarray(k_scale, dtype=np.float32)

    if "nc" not in _CACHE:
        _CACHE["nc"] = build_nc()
    nc = _CACHE["nc"]

    bf16 = ml_dtypes.bfloat16
    sgn = np.concatenate([-np.ones(64, np.float32), np.ones(64, np.float32)])
    sigma = np.concatenate([np.arange(64, 128), np.arange(0, 64)])
    identb = np.eye(128, dtype=np.float32).astype(bf16)
    onesb = np.ones((128, 128), dtype=np.float32).astype(bf16)

    def tile_sd(a):
        # [S, 128] per-batch trig -> [128 s-part, ST, 128 d]
        return np.ascontiguousarray(
            a.reshape(ST, 128, HD).transpose(1, 0, 2)).astype(bf16)

    in_maps = []
    for c in range(8):
        b, g = c // 4, c % 4
        xT = x[b].T  # [H, S]
        xti = np.ascontiguousarray(
            xT.reshape(HT, 128, ST, 128).transpose(2, 1, 0, 3)).astype(bf16)
        wq_g = Wq[:, g * 512:(g + 1) * 512]
        wk_g = Wk[:, g * 128:(g + 1) * 128]
        wv_g = Wv[:, g * 128:(g + 1) * 128]
        wqkv_g = np.concatenate([wq_g, wk_g, wv_g], axis=1)  # [H, 768]
        wqkv_g = np.ascontiguousarray(
            wqkv_g.reshape(HT, 128, 768).transpose(1, 0, 2)).astype(bf16)
        wo_g = Wo[g * 512:(g + 1) * 512, :]  # [512, H]
        wo_t = np.ascontiguousarray(
            wo_g.reshape(HPG, 128, HIDDEN).transpose(1, 0, 2)).astype(bf16)

        cosb, sinb = cos[b], sin[b]  # [S, 128]
        cq_h = cosb * q_scale[None, :]
        sq_h = (sinb * sgn[None, :]) * q_scale[sigma][None, :]
        ck_h = cosb * k_scale[None, :]
        sk_h = (sinb * sgn[None, :]) * k_scale[sigma][None, :]

        in_maps.append({
            "xt": xti,
            "wqkv": wqkv_g,
            "wo": wo_t,
            "cq": tile_sd(cq_h), "sq": tile_sd(sq_h),
            "ck": tile_sd(ck_h), "sk": tile_sd(sk_h),
            "identb": identb, "onesb": onesb,
        })

    res = run_bass_kernel_spmd(nc, in_maps, list(range(8)))
    outs = [r["y"].astype(np.float32).reshape(S, HIDDEN) for r in res.results]
    out = np.empty((B, S, HIDDEN), dtype=np.float32)
    for b in range(B):
        out[b] = (outs[4 * b] + outs[4 * b + 1]
                  + outs[4 * b + 2] + outs[4 * b + 3])
    return out
